# revision 4
# baseline (speedup 1.0000x reference)
"""Trainium2 Bass kernel for BertTempRel-style span-mean + MLP + softmax.

Reference computation (per batch row b of x[B, S, D]):
    e1 = mean(x[b, lo1:hi1, :]),  e2 = mean(x[b, lo2:hi2, :]),  cls = x[b, 0, :]
    (hi = max(hi, lo+1) empty-span guard)
    feat = concat([e1, e2, cls])            # [3D]
    out = softmax(relu(relu(feat@W1+b1)@W2+b2)@W3+b3)

Strategy: pure data-parallel over 8 NeuronCores (128 batch rows each).
Per core, x is streamed once (memory-roofline bound). The span means are
computed on the TensorEngine: for each b, the tiny 0/1 span masks (plus a
one-hot row for CLS) form the *stationary* operand [128s, 3] and the x
tile [128s, D] is the *moving* operand, accumulating [3, D] span sums in
PSUM over the 4 s-chunks (float32r mode: 1 cycle/row at N>=256). PSUM is
evacuated with a fused 1/count scale, transposed back to [d, b] layout via
tiny PE transposes, and the whole 128-row MLP runs as a handful of
matmuls at the end.
"""

import sys

if "/opt/trn_rl_repo" not in sys.path:
    sys.path.insert(0, "/opt/trn_rl_repo")

import numpy as np

from concourse import bacc, bass, mybir, tile
from concourse.bass_utils import run_bass_kernel_spmd
from concourse.masks import make_identity

F32 = mybir.dt.float32
F32R = mybir.dt.float32r
I32 = mybir.dt.int32
OP = mybir.AluOpType
AF = mybir.ActivationFunctionType

N_CORES = 8
B_FULL, S, D = 1024, 512, 768
H1, H2, H3 = 256, 64, 4
BPC = B_FULL // N_CORES  # batch rows per core (128)
BPD = 2                  # batch rows loaded per DMA


def build_program(bpc=BPC, s=S, d=D, h1=H1, h2=H2, h3=H3, bpd=BPD):
    """Emit the per-core Bass/Tile program. All 8 cores run it SPMD."""
    sc = s // 128          # s-chunks
    dh = d // 2            # moving free-dim per span matmul
    nd = d // 128          # d-chunks of 128
    nf = 3 * d // 128      # feature chunks of 128
    nh1 = h1 // 128

    nc = bacc.Bacc("TRN2", target_bir_lowering=False, debug=False,
                   num_devices=N_CORES)

    x_d = nc.dram_tensor("x", [bpc, s, d], F32R, kind="ExternalInput")
    e1_d = nc.dram_tensor("e1", [bpc, 2], I32, kind="ExternalInput")
    e2_d = nc.dram_tensor("e2", [bpc, 2], I32, kind="ExternalInput")
    w1_d = nc.dram_tensor("W1", [nf, 128, h1], F32, kind="ExternalInput")
    b1_d = nc.dram_tensor("b1", [1, h1], F32, kind="ExternalInput")
    w2_d = nc.dram_tensor("W2", [nh1, 128, h2], F32, kind="ExternalInput")
    b2_d = nc.dram_tensor("b2", [1, h2], F32, kind="ExternalInput")
    w3_d = nc.dram_tensor("W3", [h2, h3], F32, kind="ExternalInput")
    b3_d = nc.dram_tensor("b3", [1, h3], F32, kind="ExternalInput")
    out_d = nc.dram_tensor("out", [bpc, h3], F32, kind="ExternalOutput")

    with tile.TileContext(nc) as tc:
        with tc.tile_pool(name="const", bufs=1) as const:
            ident = const.tile([128, 128], F32)
            make_identity(nc, ident[:])

            w1 = const.tile([128, nf, h1], F32)
            nc.sync.dma_start(w1[:], w1_d.ap().rearrange("p k h -> k p h"))
            w2 = const.tile([128, nh1, h2], F32)
            nc.sync.dma_start(w2[:], w2_d.ap().rearrange("p k h -> k p h"))
            w3 = const.tile([h2, h3], F32)
            nc.sync.dma_start(w3[:], w3_d.ap()[:])
            b1r = const.tile([1, h1], F32)
            nc.sync.dma_start(b1r[:], b1_d.ap()[:])
            b2r = const.tile([1, h2], F32)
            nc.sync.dma_start(b2r[:], b2_d.ap()[:])
            b3r = const.tile([1, h3], F32)
            nc.sync.dma_start(b3r[:], b3_d.ap()[:])
            ones = const.tile([1, 128], F32)
            nc.vector.memset(ones[:], 1.0)

            # ---- span bounds, counts, reciprocal counts ([b, *] layout) ----
            sp_i = const.tile([bpc, 4], I32)
            nc.sync.dma_start(sp_i[:, 0:2], e1_d.ap()[:])
            nc.sync.dma_start(sp_i[:, 2:4], e2_d.ap()[:])
            sp_f = const.tile([bpc, 4], F32)
            nc.vector.tensor_copy(sp_f[:], sp_i[:])

            bounds = const.tile([bpc, 4], F32)  # lo1, hi1, lo2, hi2 (guarded)
            rp = const.tile([bpc, 3], F32)      # 1/cnt1, 1/cnt2, 1.0
            cnt = const.tile([bpc, 2], F32)
            for j in range(2):
                lo = sp_f[:, 2 * j:2 * j + 1]
                hi_raw = sp_f[:, 2 * j + 1:2 * j + 2]
                lo_out = bounds[:, 2 * j:2 * j + 1]
                hi_out = bounds[:, 2 * j + 1:2 * j + 2]
                nc.vector.tensor_copy(lo_out, lo)
                # hi = max(hi_raw, lo+1); cnt = hi - lo; rp = 1/cnt
                nc.vector.tensor_scalar(hi_out, lo, 1.0, None, OP.add)
                nc.vector.tensor_tensor(hi_out, hi_raw, hi_out, OP.max)
                nc.vector.tensor_tensor(cnt[:, j:j + 1], hi_out, lo_out,
                                        OP.subtract)
                nc.vector.reciprocal(rp[:, j:j + 1], cnt[:, j:j + 1])
            nc.vector.memset(rp[:, 2:3], 1.0)

            # ---- masks in [b, s] layout ----
            iota_i = const.tile([bpc, s], I32)
            nc.gpsimd.iota(iota_i[:], pattern=[[1, s]], base=0,
                           channel_multiplier=0)
            iota_f = const.tile([bpc, s], F32)
            nc.vector.tensor_copy(iota_f[:], iota_i[:])

            masks = const.tile([bpc, 3, s], F32)
            ge = const.tile([bpc, s], F32)
            for j in range(2):
                lo = bounds[:, 2 * j:2 * j + 1]
                hi = bounds[:, 2 * j + 1:2 * j + 2]
                nc.vector.tensor_scalar(ge[:], iota_f[:], lo, None, OP.is_ge)
                nc.vector.scalar_tensor_tensor(masks[:, j, :], iota_f[:], hi,
                                               ge[:], OP.is_lt, OP.mult)
            # CLS one-hot "mask": 1.0 at s == 0
            nc.vector.tensor_scalar(masks[:, 2, :], iota_f[:], 0.0, None,
                                    OP.is_equal)

            # ---- transpose masks/scales to [s, b] / [3, b] layouts ----
            # mt[s_p, c, b, m]: stationary operand source; m: e1, e2, cls.
            mt = const.tile([128, sc, bpc, 3], F32R)
            scl = const.tile([3, bpc], F32)
            with tc.tile_pool(name="p0psum", bufs=2, space="PSUM") as p0p:
                for c in range(sc):
                    for j in range(3):
                        tp = p0p.tile([128, bpc], F32, tag="tp")
                        nc.tensor.transpose(tp[:], masks[:, j, bass.ts(c, 128)],
                                            ident[0:bpc, 0:bpc])
                        nc.vector.tensor_copy(mt[:, c, :, j], tp[:])
                tps = p0p.tile([3, bpc], F32, tag="tps")
                nc.tensor.transpose(tps[:], rp[:], ident[0:bpc, 0:bpc])
                nc.vector.tensor_copy(scl[:], tps[:])

            # packT[d_p, dc, b, m]: transposed scaled span sums / cls.
            packT = const.tile([128, nd, bpc, 3], F32)

            # ---- main loop: stream x, accumulate span sums on PE ----
            with tc.tile_pool(name="xp", bufs=3) as xp, \
                 tc.tile_pool(name="stg", bufs=4) as stg, \
                 tc.tile_pool(name="sps0", bufs=2, space="PSUM") as sps0, \
                 tc.tile_pool(name="sps1", bufs=2, space="PSUM") as sps1, \
                 tc.tile_pool(name="ptp", bufs=2, space="PSUM") as ptp:
                for i in range(bpc // bpd):
                    xb = xp.tile([128, bpd, sc, d], F32R, tag="xb")
                    dma_eng = nc.sync if i % 2 == 0 else nc.scalar
                    dma_eng.dma_start(
                        xb[:],
                        x_d.ap()[bpd * i:bpd * (i + 1)].rearrange(
                            "b (c p) d -> p b c d", p=128))
                    for j in range(bpd):
                        b = bpd * i + j
                        ps0 = sps0.tile([3, dh], F32, tag="ps0")
                        ps1 = sps1.tile([3, dh], F32, tag="ps1")
                        for c in range(sc):
                            lhsT = mt[:, c, b, :]
                            nc.tensor.matmul(ps0[:], lhsT,
                                             xb[:, j, c, 0:dh],
                                             start=(c == 0), stop=(c == sc - 1))
                            nc.tensor.matmul(ps1[:], lhsT,
                                             xb[:, j, c, dh:d],
                                             start=(c == 0), stop=(c == sc - 1))
                        # evacuate + scale by 1/cnt (split across DVE/ACT)
                        sg = stg.tile([3, d], F32, tag="sg")
                        nc.vector.tensor_scalar(sg[:, 0:dh], ps0[:],
                                                scl[:, b:b + 1], None, OP.mult)
                        nc.scalar.mul(sg[:, dh:d], ps1[:], scl[:, b:b + 1])
                        # transpose [3, d] -> nd x [128, 3] columns of packT
                        for dc in range(nd):
                            pt = ptp.tile([128, 3], F32, tag="pt")
                            nc.tensor.transpose(pt[:], sg[:, bass.ts(dc, 128)],
                                                ident[0:3, 0:3])
                            if dc % 2 == 0:
                                nc.vector.tensor_copy(packT[:, dc, b, :], pt[:])
                            else:
                                nc.scalar.copy(packT[:, dc, b, :], pt[:])

            # ---- de-interleave features: featT[f_p, p, b] ----
            featT = const.tile([128, nf, bpc], F32)
            for m in range(3):
                for dc in range(nd):
                    nc.vector.tensor_copy(featT[:, m * nd + dc, :],
                                          packT[:, dc, :, m])

            # ---- MLP + softmax over all bpc rows at once ----
            h1s = const.tile([bpc, h1], F32)
            h1T = const.tile([128, nh1, bpc], F32)
            h2s = const.tile([bpc, h2], F32)
            h2T = const.tile([h2, bpc], F32)
            probs = const.tile([bpc, h3], F32)
            mx = const.tile([bpc, 1], F32)
            ex = const.tile([bpc, h3], F32)
            sm = const.tile([bpc, 1], F32)
            rc = const.tile([bpc, 1], F32)

            with tc.tile_pool(name="mlpp", bufs=1, space="PSUM") as mp:
                h1p = mp.tile([bpc, h1], F32, tag="h1p")
                for p in range(nf):
                    nc.tensor.matmul(h1p[:], featT[:, p, :], w1[:, p, :],
                                     start=(p == 0), stop=False)
                nc.tensor.matmul(h1p[:], ones[0:1, 0:bpc], b1r[:],
                                 start=False, stop=True)
                nc.scalar.activation(h1s[:], h1p[:], AF.Relu)

                for k in range(nh1):
                    tp1 = mp.tile([128, bpc], F32, tag="tp1")
                    nc.tensor.transpose(tp1[:], h1s[:, bass.ts(k, 128)],
                                        ident[0:bpc, 0:bpc])
                    nc.vector.tensor_copy(h1T[:, k, :], tp1[:])

                h2p = mp.tile([bpc, h2], F32, tag="h2p")
                for k in range(nh1):
                    nc.tensor.matmul(h2p[:], h1T[:, k, :], w2[:, k, :],
                                     start=(k == 0), stop=False)
                nc.tensor.matmul(h2p[:], ones[0:1, 0:bpc], b2r[:],
                                 start=False, stop=True)
                nc.scalar.activation(h2s[:], h2p[:], AF.Relu)

                tp2 = mp.tile([h2, bpc], F32, tag="tp2")
                nc.tensor.transpose(tp2[:], h2s[:], ident[0:bpc, 0:bpc])
                nc.vector.tensor_copy(h2T[:], tp2[:])

                h3p = mp.tile([bpc, h3], F32, tag="h3p")
                nc.tensor.matmul(h3p[:], h2T[:], w3[:], start=True, stop=False)
                nc.tensor.matmul(h3p[:], ones[0:1, 0:bpc], b3r[:],
                                 start=False, stop=True)

                # softmax along the 4 logits
                nc.vector.tensor_reduce(mx[:], h3p[:], mybir.AxisListType.X,
                                        OP.max, negate=True)
                nc.scalar.activation(ex[:], h3p[:], AF.Exp, bias=mx[:],
                                     scale=1.0)
                nc.vector.tensor_reduce(sm[:], ex[:], mybir.AxisListType.X,
                                        OP.add)
                nc.vector.reciprocal(rc[:], sm[:])
                nc.vector.tensor_scalar(probs[:], ex[:], rc[:], None, OP.mult)

            nc.sync.dma_start(out_d.ap()[:], probs[:])

    nc.compile()
    return nc


_NC_CACHE = {}


def _get_program():
    if "nc" not in _NC_CACHE:
        _NC_CACHE["nc"] = build_program()
    return _NC_CACHE["nc"]


def make_in_maps(inputs):
    x = np.ascontiguousarray(np.asarray(inputs["x"], dtype=np.float32))
    e1 = np.ascontiguousarray(np.asarray(inputs["e1_span"], dtype=np.int32))
    e2 = np.ascontiguousarray(np.asarray(inputs["e2_span"], dtype=np.int32))
    w1 = np.ascontiguousarray(
        np.asarray(inputs["W1"], dtype=np.float32).reshape(3 * D // 128, 128, H1))
    b1 = np.asarray(inputs["b1"], dtype=np.float32).reshape(1, H1)
    w2 = np.ascontiguousarray(
        np.asarray(inputs["W2"], dtype=np.float32).reshape(H1 // 128, 128, H2))
    b2 = np.asarray(inputs["b2"], dtype=np.float32).reshape(1, H2)
    w3 = np.ascontiguousarray(np.asarray(inputs["W3"], dtype=np.float32))
    b3 = np.asarray(inputs["b3"], dtype=np.float32).reshape(1, H3)

    in_maps = []
    for c in range(N_CORES):
        sl = slice(c * BPC, (c + 1) * BPC)
        in_maps.append({
            "x": np.ascontiguousarray(x[sl]),
            "e1": np.ascontiguousarray(e1[sl]),
            "e2": np.ascontiguousarray(e2[sl]),
            "W1": w1, "b1": b1, "W2": w2, "b2": b2, "W3": w3, "b3": b3,
        })
    return in_maps


def kernel(**inputs) -> np.ndarray:
    nc = _get_program()
    res = run_bass_kernel_spmd(nc, make_in_maps(inputs),
                               core_ids=list(range(N_CORES)))
    return np.concatenate([res.results[c]["out"] for c in range(N_CORES)],
                          axis=0)
